# revision 1
# baseline (speedup 1.0000x reference)
"""PreT_Attention (prefix-KV multi-head attention) on 8 Trainium2 NeuronCores.

Strategy: pure data parallelism — batch B=8 is split 1 element per core; the
qkv/proj weights are replicated. No collectives. Host-side numpy does layout
marshalling only (transposes / reshapes), all FLOPs run on device.

Device kernel (per core), all fp32 data with float32r matmuls:
  1. qkv projection from pre-transposed x^T and W^T (contraction dim c on
     partitions): q^T,k^T come out head-transposed (d,n); v comes out natural
     (n,d) with an interleaved ones column per head for softmax denominators.
  2. S^T = k^T.T @ q^T per (head, m-tile) -> exp on ScalarE (scale=1/8 folded
     in) -> O_aug = [v|1].T @ E accumulated over m-tiles in PSUM; row 64 of
     O_aug is the softmax denominator row.
  3. normalize via DVE reciprocal + gpsimd partition_broadcast + DVE multiply
     into A^T (c,n), then output projection from A^T and W_proj^T with the
     bias added via a ones-row matmul. y is written in natural (n,c) layout.

The m (key/value position) axis is ordered [tokens(1024) | prefix(64)] —
softmax is permutation invariant, and this keeps every tile 128-aligned.
"""

import os
import sys

if os.environ.get("PRET_NOCACHE"):
    try:
        import jax
        jax.config.update("jax_enable_compilation_cache", False)
    except Exception:
        pass

for _p in ("/opt/trn_rl_repo", "/root/.axon_site/_ro/trn_rl_repo"):
    if os.path.isdir(_p) and _p not in sys.path:
        sys.path.insert(0, _p)

import numpy as np

import contextlib

import concourse.bass as bass
import concourse.mybir as mybir
import concourse.tile as tile
from concourse import bacc
from concourse import bass_utils
from concourse import library_config
from concourse.bass_utils import run_bass_kernel_spmd


@contextlib.contextmanager
def _ldw_opt():
    # walrus defaults to --enable-ldw-opt=false; enabling it overlaps the
    # (otherwise serialized) fp32r weight loads and saves ~130us/kernel
    orig = bass_utils.run_command

    def patched(argv, **kw):
        argv = ["--enable-ldw-opt=true" if a == "--enable-ldw-opt=false" else a for a in argv]
        return orig(argv, **kw)

    bass_utils.run_command = patched
    try:
        yield
    finally:
        bass_utils.run_command = orig

F32 = mybir.dt.float32
R32 = mybir.dt.float32r
BF16 = mybir.dt.bfloat16
EXP = mybir.ActivationFunctionType.Exp

B, N, C, H, D, P = 8, 1024, 768, 12, 64, 64
M = N + P            # 1088 key/value positions, tokens first then prefix
KT = C // 128        # 6 contraction k-tiles
NT = N // 128        # 8 token tiles
MT = M // 128        # 8 full m-tiles (+1 half tile for the prefix)
SCALE = D ** -0.5

# epilogue broadcast method: "gpsimd" (partition_broadcast) or "matmul"
BCAST = os.environ.get("PRET_BCAST", "gpsimd")
# perf-measurement knob: emit the whole computation REPEAT times so the
# marginal cost per repeat isolates device time from dispatch overhead
REPEAT = int(os.environ.get("PRET_REPEAT", "1"))
# matmul operand dtype: fp32r (rounded fp32, full-rate) or bf16
DT = {"fp32r": R32, "bf16": BF16}[os.environ.get("PRET_DT", "fp32r")]


def build_nc(repeat=REPEAT):
    nc = bacc.Bacc("TRN2", target_bir_lowering=False, debug=False)

    xT = nc.dram_tensor("xT", (KT, 128, N), DT, kind="ExternalInput")
    wq = nc.dram_tensor("wq", (KT, 128, 3 * C), DT, kind="ExternalInput")
    wp = nc.dram_tensor("wp", (KT, 128, C), DT, kind="ExternalInput")
    kp = nc.dram_tensor("kp", (H, D, P), DT, kind="ExternalInput")
    vp = nc.dram_tensor("vp", (H, P, D), DT, kind="ExternalInput")
    bi = nc.dram_tensor("bi", (1, C), DT, kind="ExternalInput")
    y = nc.dram_tensor("y", (NT, 128, C), F32, kind="ExternalOutput")

    with tile.TileContext(nc) as tc:
        with (
            nc.allow_low_precision(reason="fp32r is a rounded fp32 used for full-rate matmuls"),
            tc.tile_pool(name="const", bufs=1) as const_pool,
            tc.tile_pool(name="data", bufs=1) as data_pool,
            tc.tile_pool(name="work", bufs=2) as work_pool,
            tc.tile_pool(name="psum", bufs=2, space="PSUM") as pp,
        ):
            # ---- persistent SBUF tensors ----
            bi_sb = const_pool.tile([1, C], DT)
            ones_sb = const_pool.tile([1, 128], DT)

            q_sb = data_pool.tile([128, KT, N], DT)          # q^T, pair rows
            kall_sb = data_pool.tile([128, KT, M], DT)       # k^T, pair rows
            v_sb = data_pool.tile([128, MT + 1, H * 65], DT)  # v + ones cols

            if BCAST == "gpsimd":
                nc.gpsimd.load_library(library_config.attn)
            # memset can't write fp32r; stage ones in f32 and copy (rounds)
            ones_f32 = const_pool.tile([128, 128], F32)
            nc.vector.memset(ones_f32[:], 1.0)
            nc.vector.tensor_copy(ones_sb[:], ones_f32[0:1, :])
            # col 64 of each head block must be 1.0 (softmax denominators)
            v_ones = v_sb.rearrange("p m (h e) -> p m h e", e=65)[:, :, :, 64]
            nc.vector.tensor_copy(
                v_ones, ones_f32[:, 0 : (MT + 1) * H].rearrange("p (m h) -> p m h", m=MT + 1)
            )

            nc.sync.dma_start(bi_sb[:], bi[:])
            # prefix k^T -> kall cols [N, N+P) ; heads stacked per pair tile
            for t in range(KT):
                nc.sync.dma_start(
                    kall_sb[:, t, N:M],
                    kp[2 * t : 2 * t + 2].rearrange("h d p -> (h d) p"),
                )
            # prefix v (natural) -> v_sb m-tile MT, interleaved head blocks
            v_pre = v_sb.rearrange("p m (h e) -> p m h e", e=65)[0:P, MT, :, 0:D]
            nc.sync.dma_start(v_pre, vp.rearrange("h p d -> p h d"))

            def emit_body():
              # ---- phase 1: qkv projections ----
              p1_pool = tc.alloc_tile_pool(name="p1", bufs=1)
              wq_sb = p1_pool.tile([128, KT, 3 * C], DT)
              xT_sb = p1_pool.tile([128, KT, N], DT)
              for kt in range(KT):
                  nc.sync.dma_start(xT_sb[:, kt, :], xT[kt])
              for kt in range(KT):
                  nc.sync.dma_start(wq_sb[:, kt, :], wq[kt])

              def emit_qk_tile(mt):
                  # M-tile mt of [q^T; k^T] (rows j = mt*128..): stationary W^T
                  ps = pp.tile([128, N], F32, tag="s", name=f"ps_qk{mt}")
                  for kt in range(KT):
                      for nb in range(2):
                          nc.tensor.matmul(
                              ps[:, nb * 512 : (nb + 1) * 512],
                              wq_sb[:, kt, mt * 128 : (mt + 1) * 128],
                              xT_sb[:, kt, nb * 512 : (nb + 1) * 512],
                              start=(kt == 0),
                              stop=(kt == KT - 1),
                          )
                  if mt < KT:
                      nc.vector.tensor_copy(q_sb[:, mt, :], ps[:])
                  else:
                      nc.vector.tensor_copy(kall_sb[:, mt - KT, 0:N], ps[:])

              def emit_v_tile(nt):
                  # n-tile nt of natural v: stationary x^T, moving W_v^T
                  ps = pp.tile([128, 1024], F32, tag="o", name=f"ps_v{nt}")
                  for kt in range(KT):
                      for j0, j1 in ((0, 512), (512, C)):
                          nc.tensor.matmul(
                              ps[:, j0:j1],
                              xT_sb[:, kt, nt * 128 : (nt + 1) * 128],
                              wq_sb[:, kt, 2 * C + j0 : 2 * C + j1],
                              start=(kt == 0),
                              stop=(kt == KT - 1),
                          )
                  dst = v_sb.rearrange("p m (h e) -> p m h e", e=65)[:, nt, :, 0:D]
                  nc.vector.tensor_copy(dst, ps[:, 0:C].rearrange("p (h d) -> p h d", h=H))

              # order so head 0's operands (k pair 0, q pair 0, v tiles) are ready early
              v_order = iter(range(NT))
              for i in range(KT):
                  emit_qk_tile(KT + i)   # k pair i
                  emit_qk_tile(i)        # q pair i
                  emit_v_tile(next(v_order))
                  if i in (0, 2):
                      emit_v_tile(next(v_order))
              p1_pool.release()

              # phase-3 operands (space reuses the released p1 pool's range)
              p3_pool = tc.alloc_tile_pool(name="p3", bufs=1)
              wp_sb = p3_pool.tile([128, KT, C], DT)
              a_sb = p3_pool.tile([128, KT, N], DT)            # A^T attn out
              for kt in range(KT):
                  nc.sync.dma_start(wp_sb[:, kt, :], wp[kt])

              # ---- phase 2: attention per head ----
              for h in range(H):
                  t, r = h // 2, (h % 2) * 64
                  po = pp.tile([65, N], F32, tag="o", name=f"ps_o{h}")
                  for mt in range(MT + 1):
                      mw = 128 if mt < MT else P
                      ps = pp.tile([mw, N], F32, tag="s", name=f"ps_s{h}_{mt}")
                      for nb in range(2):
                          nc.tensor.matmul(
                              ps[:, nb * 512 : (nb + 1) * 512],
                              kall_sb[r : r + D, t, mt * 128 : mt * 128 + mw],
                              q_sb[r : r + D, t, nb * 512 : (nb + 1) * 512],
                              start=True,
                              stop=True,
                          )
                      e_sb = work_pool.tile([mw, N], DT, tag="e", bufs=3, name=f"e{h}_{mt}")
                      nc.scalar.activation(e_sb[:], ps[:], EXP, scale=SCALE)
                      for nb in range(2):
                          nc.tensor.matmul(
                              po[:, nb * 512 : (nb + 1) * 512],
                              v_sb[0:mw, mt, h * 65 : (h + 1) * 65],
                              e_sb[:, nb * 512 : (nb + 1) * 512],
                              start=(mt == 0),
                              stop=(mt == MT),
                          )
                  # normalize rows 0..63 by row 64 (denominators), write A^T
                  r_sb = work_pool.tile([1, N], DT if BCAST == "matmul" else F32, tag="r", name=f"r{h}")
                  nc.vector.reciprocal(r_sb[:], po[64:65, :])
                  if BCAST == "gpsimd":
                      rb_sb = work_pool.tile([64, N], F32, tag="rb", name=f"rb{h}")
                      nc.gpsimd.partition_broadcast(rb_sb[:], r_sb[:])
                      nc.vector.tensor_mul(a_sb[r : r + 64, t, :], po[0:64, :], rb_sb[:])
                  else:
                      o_sb = work_pool.tile([64, N], F32, tag="osb", name=f"o{h}")
                      nc.vector.tensor_copy(o_sb[:], po[0:64, :])
                      rb_ps = pp.tile([64, N], F32, tag="s", name=f"rb{h}")
                      for nb in range(2):
                          nc.tensor.matmul(
                              rb_ps[:, nb * 512 : (nb + 1) * 512],
                              ones_sb[0:1, 0:64],
                              r_sb[0:1, nb * 512 : (nb + 1) * 512],
                              start=True,
                              stop=True,
                          )
                      nc.vector.tensor_mul(a_sb[r : r + 64, t, :], o_sb[:], rb_ps[:])

              # ---- phase 3: output projection + bias ----
              for nt in range(NT):
                  py = pp.tile([128, 1024], F32, tag="s", name=f"ps_y{nt}")
                  for kt in range(KT):
                      for j0, j1 in ((0, 512), (512, C)):
                          nc.tensor.matmul(
                              py[:, j0:j1],
                              a_sb[:, kt, nt * 128 : (nt + 1) * 128],
                              wp_sb[:, kt, j0:j1],
                              start=(kt == 0),
                              stop=False,
                          )
                  for j0, j1 in ((0, 512), (512, C)):
                      nc.tensor.matmul(
                          py[:, j0:j1],
                          ones_sb[0:1, :],
                          bi_sb[0:1, j0:j1],
                          start=False,
                          stop=True,
                      )
                  y_sb = work_pool.tile([128, C], F32, tag="y", name=f"y{nt}")
                  nc.vector.tensor_copy(y_sb[:], py[:, 0:C])
                  nc.sync.dma_start(y[nt], y_sb[:])
              p3_pool.release()

            for _ in range(repeat):
                emit_body()

    nc.compile()
    return nc


def make_in_maps(x, prompt, W_qkv, W_proj, b_proj):
    if DT == BF16:
        import ml_dtypes
        cast = lambda a: np.asarray(a, dtype=ml_dtypes.bfloat16)
    else:
        cast = lambda a: np.asarray(a, dtype=np.float32)
    x, prompt, W_qkv, W_proj, b_proj = map(cast, (x, prompt, W_qkv, W_proj, b_proj))
    wq_h = np.ascontiguousarray(W_qkv.T.reshape(KT, 128, 3 * C))
    wp_h = np.ascontiguousarray(W_proj.T.reshape(KT, 128, C))
    bi_h = np.ascontiguousarray(b_proj.reshape(1, C))
    maps = []
    for b in range(B):
        maps.append(
            {
                "xT": np.ascontiguousarray(x[b].T.reshape(KT, 128, N)),
                "wq": wq_h,
                "wp": wp_h,
                "kp": np.ascontiguousarray(prompt[b, 0].transpose(1, 2, 0)),
                "vp": np.ascontiguousarray(prompt[b, 1].transpose(1, 0, 2)),
                "bi": bi_h,
            }
        )
    return maps


def kernel(x, prompt, W_qkv, W_proj, b_proj, **run_kwargs):
    x = np.asarray(x, dtype=np.float32)
    prompt = np.asarray(prompt, dtype=np.float32)
    W_qkv = np.asarray(W_qkv, dtype=np.float32)
    W_proj = np.asarray(W_proj, dtype=np.float32)
    b_proj = np.asarray(b_proj, dtype=np.float32)

    nc = build_nc(repeat=1)
    in_maps = make_in_maps(x, prompt, W_qkv, W_proj, b_proj)
    with _ldw_opt():
        res = run_bass_kernel_spmd(nc, in_maps, core_ids=list(range(B)), **run_kwargs)
    out = np.stack([res.results[b]["y"].reshape(N, C) for b in range(B)])
    if run_kwargs:
        kernel.last_results = res
    return out



# revision 4
# speedup vs baseline: 4.3437x; 4.3437x over previous
"""PreT_Attention (prefix-KV multi-head attention) on 8 Trainium2 NeuronCores.

Strategy: pure data parallelism — batch B=8 is split 1 element per core; the
qkv/proj weights are replicated. No collectives. Host-side numpy does layout
marshalling only (transposes / reshapes), all FLOPs run on device.

Device kernel (per core), all fp32 data with float32r matmuls:
  1. qkv projection from pre-transposed x^T and W^T (contraction dim c on
     partitions): q^T,k^T come out head-transposed (d,n); v comes out natural
     (n,d) with an interleaved ones column per head for softmax denominators.
  2. S^T = k^T.T @ q^T per (head, m-tile) -> exp on ScalarE (scale=1/8 folded
     in) -> O_aug = [v|1].T @ E accumulated over m-tiles in PSUM; row 64 of
     O_aug is the softmax denominator row.
  3. normalize via DVE reciprocal + gpsimd partition_broadcast + DVE multiply
     into A^T (c,n), then output projection from A^T and W_proj^T with the
     bias added via a ones-row matmul. y is written in natural (n,c) layout.

The m (key/value position) axis is ordered [tokens(1024) | prefix(64)] —
softmax is permutation invariant, and this keeps every tile 128-aligned.
"""

import os
import sys

if os.environ.get("PRET_NOCACHE"):
    try:
        import jax
        jax.config.update("jax_enable_compilation_cache", False)
    except Exception:
        pass

for _p in ("/opt/trn_rl_repo", "/root/.axon_site/_ro/trn_rl_repo"):
    if os.path.isdir(_p) and _p not in sys.path:
        sys.path.insert(0, _p)

import numpy as np

import contextlib

import concourse.bass as bass
import concourse.mybir as mybir
import concourse.tile as tile
from concourse import bacc
from concourse import bass_utils
from concourse import library_config
from concourse.bass_utils import run_bass_kernel_spmd


@contextlib.contextmanager
def _ldw_opt():
    # walrus defaults to --enable-ldw-opt=false; enabling it overlaps the
    # (otherwise serialized) fp32r weight loads and saves ~130us/kernel.
    # bf16 matmuls emit explicit InstLdweights that walrus rejects under
    # ldw-opt ("InstLdweights is not compatible with LDW optimization"),
    # so only patch the flag for fp32r builds.
    if DT != R32:
        yield
        return
    orig = bass_utils.run_command

    def patched(argv, **kw):
        argv = ["--enable-ldw-opt=true" if a == "--enable-ldw-opt=false" else a for a in argv]
        return orig(argv, **kw)

    bass_utils.run_command = patched
    try:
        yield
    finally:
        bass_utils.run_command = orig

F32 = mybir.dt.float32
R32 = mybir.dt.float32r
BF16 = mybir.dt.bfloat16
EXP = mybir.ActivationFunctionType.Exp

B, N, C, H, D, P = 8, 1024, 768, 12, 64, 64
M = N + P            # 1088 key/value positions, tokens first then prefix
KT = C // 128        # 6 contraction k-tiles
NT = N // 128        # 8 token tiles
MT = M // 128        # 8 full m-tiles (+1 half tile for the prefix)
SCALE = D ** -0.5

# epilogue broadcast method: "gpsimd" (partition_broadcast) or "matmul"
BCAST = os.environ.get("PRET_BCAST", "gpsimd")
# perf-measurement knob: emit the whole computation REPEAT times so the
# marginal cost per repeat isolates device time from dispatch overhead
REPEAT = int(os.environ.get("PRET_REPEAT", "1"))
# matmul operand dtype: fp32r (rounded fp32, full-rate) or bf16.
# bf16 halves DMA traffic and SBUF footprint at the same matmul rate
# (1 cycle/row either way); rel err ~5e-3 vs the 2e-2 gate.
DT = {"fp32r": R32, "bf16": BF16}[os.environ.get("PRET_DT", "bf16")]


def build_nc(repeat=REPEAT):
    nc = bacc.Bacc("TRN2", target_bir_lowering=False, debug=False)

    xT = nc.dram_tensor("xT", (KT, 128, N), DT, kind="ExternalInput")
    wq = nc.dram_tensor("wq", (KT, 128, 3 * C), DT, kind="ExternalInput")
    wp = nc.dram_tensor("wp", (KT, 128, C), DT, kind="ExternalInput")
    kp = nc.dram_tensor("kp", (H, D, P), DT, kind="ExternalInput")
    vp = nc.dram_tensor("vp", (H, P, D), DT, kind="ExternalInput")
    bi = nc.dram_tensor("bi", (1, C), DT, kind="ExternalInput")
    y = nc.dram_tensor("y", (NT, 128, C), F32, kind="ExternalOutput")

    with tile.TileContext(nc) as tc:
        with (
            nc.allow_low_precision(reason="fp32r is a rounded fp32 used for full-rate matmuls"),
            tc.tile_pool(name="const", bufs=1) as const_pool,
            tc.tile_pool(name="data", bufs=1) as data_pool,
            tc.tile_pool(name="work", bufs=2) as work_pool,
            tc.tile_pool(name="psum", bufs=2, space="PSUM") as pp,
        ):
            # ---- persistent SBUF tensors ----
            bi_sb = const_pool.tile([1, C], DT)
            ones_sb = const_pool.tile([1, 128], DT)

            q_sb = data_pool.tile([128, KT, N], DT)          # q^T, pair rows
            kall_sb = data_pool.tile([128, KT, M], DT)       # k^T, pair rows
            v_sb = data_pool.tile([128, MT + 1, H * 65], DT)  # v + ones cols

            if BCAST == "gpsimd":
                nc.gpsimd.load_library(library_config.attn)
            # memset can't write fp32r; stage ones in f32 and copy (rounds)
            ones_f32 = const_pool.tile([128, 128], F32)
            nc.vector.memset(ones_f32[:], 1.0)
            nc.vector.tensor_copy(ones_sb[:], ones_f32[0:1, :])
            # col 64 of each head block must be 1.0 (softmax denominators)
            v_ones = v_sb.rearrange("p m (h e) -> p m h e", e=65)[:, :, :, 64]
            nc.vector.tensor_copy(
                v_ones, ones_f32[:, 0 : (MT + 1) * H].rearrange("p (m h) -> p m h", m=MT + 1)
            )

            nc.sync.dma_start(bi_sb[:], bi[:])
            # prefix k^T -> kall cols [N, N+P) ; heads stacked per pair tile
            for t in range(KT):
                nc.sync.dma_start(
                    kall_sb[:, t, N:M],
                    kp[2 * t : 2 * t + 2].rearrange("h d p -> (h d) p"),
                )
            # prefix v (natural) -> v_sb m-tile MT, interleaved head blocks
            v_pre = v_sb.rearrange("p m (h e) -> p m h e", e=65)[0:P, MT, :, 0:D]
            nc.sync.dma_start(v_pre, vp.rearrange("h p d -> p h d"))

            def emit_body():
              # ---- phase 1: qkv projections ----
              # xT / wq live in per-kt tiles so write->read deps are per
              # chunk (a single fused tile would stall the first matmul on
              # ALL 12 DMAs); each out-tile starts its psum accumulation at
              # a rotated kt so matmuls unlock progressively as chunks land.
              p1_pool = tc.alloc_tile_pool(name="p1", bufs=1)
              wq_sb = [p1_pool.tile([128, 3 * C], DT, name=f"wq{kt}") for kt in range(KT)]
              xT_sb = [p1_pool.tile([128, N], DT, name=f"xt{kt}") for kt in range(KT)]
              for kt in range(KT):
                  nc.sync.dma_start(xT_sb[kt][:], xT[kt])
                  nc.sync.dma_start(wq_sb[kt][:], wq[kt])

              tile_seq = iter(range(64))

              def emit_qk_tile(mt):
                  # M-tile mt of [q^T; k^T] (rows j = mt*128..): stationary W^T
                  ps = pp.tile([128, N], F32, tag="s", name=f"ps_qk{mt}")
                  k0 = next(tile_seq)
                  for i in range(KT):
                      kt = (k0 + i) % KT
                      for nb in range(2):
                          nc.tensor.matmul(
                              ps[:, nb * 512 : (nb + 1) * 512],
                              wq_sb[kt][:, mt * 128 : (mt + 1) * 128],
                              xT_sb[kt][:, nb * 512 : (nb + 1) * 512],
                              start=(i == 0),
                              stop=(i == KT - 1),
                          )
                  if mt < KT:
                      nc.vector.tensor_copy(q_sb[:, mt, :], ps[:])
                  else:
                      nc.vector.tensor_copy(kall_sb[:, mt - KT, 0:N], ps[:])

              def emit_v_tile(nt):
                  # n-tile nt of natural v: stationary x^T, moving W_v^T
                  ps = pp.tile([128, 1024], F32, tag="o", name=f"ps_v{nt}")
                  k0 = next(tile_seq)
                  for i in range(KT):
                      kt = (k0 + i) % KT
                      for j0, j1 in ((0, 512), (512, C)):
                          nc.tensor.matmul(
                              ps[:, j0:j1],
                              xT_sb[kt][:, nt * 128 : (nt + 1) * 128],
                              wq_sb[kt][:, 2 * C + j0 : 2 * C + j1],
                              start=(i == 0),
                              stop=(i == KT - 1),
                          )
                  dst = v_sb.rearrange("p m (h e) -> p m h e", e=65)[:, nt, :, 0:D]
                  nc.vector.tensor_copy(dst, ps[:, 0:C].rearrange("p (h d) -> p h d", h=H))

              # order so head 0's operands (k pair 0, q pair 0, v tiles) are ready early
              v_order = iter(range(NT))
              for i in range(KT):
                  emit_qk_tile(KT + i)   # k pair i
                  emit_qk_tile(i)        # q pair i
                  emit_v_tile(next(v_order))
                  if i in (0, 2):
                      emit_v_tile(next(v_order))
              p1_pool.release()

              # phase-3 operands (space reuses the released p1 pool's range)
              p3_pool = tc.alloc_tile_pool(name="p3", bufs=1)
              wp_sb = p3_pool.tile([128, KT, C], DT)
              a_sb = p3_pool.tile([128, KT, N], DT)            # A^T attn out
              for kt in range(KT):
                  nc.sync.dma_start(wp_sb[:, kt, :], wp[kt])

              # ---- phase 2: attention per head ----
              for h in range(H):
                  t, r = h // 2, (h % 2) * 64
                  po = pp.tile([65, N], F32, tag="o", name=f"ps_o{h}")
                  for mt in range(MT + 1):
                      mw = 128 if mt < MT else P
                      ps = pp.tile([mw, N], F32, tag="s", name=f"ps_s{h}_{mt}")
                      for nb in range(2):
                          nc.tensor.matmul(
                              ps[:, nb * 512 : (nb + 1) * 512],
                              kall_sb[r : r + D, t, mt * 128 : mt * 128 + mw],
                              q_sb[r : r + D, t, nb * 512 : (nb + 1) * 512],
                              start=True,
                              stop=True,
                          )
                      e_sb = work_pool.tile([mw, N], DT, tag="e", bufs=3, name=f"e{h}_{mt}")
                      nc.scalar.activation(e_sb[:], ps[:], EXP, scale=SCALE)
                      for nb in range(2):
                          nc.tensor.matmul(
                              po[:, nb * 512 : (nb + 1) * 512],
                              v_sb[0:mw, mt, h * 65 : (h + 1) * 65],
                              e_sb[:, nb * 512 : (nb + 1) * 512],
                              start=(mt == 0),
                              stop=(mt == MT),
                          )
                  # normalize rows 0..63 by row 64 (denominators), write A^T
                  r_sb = work_pool.tile([1, N], DT if BCAST == "matmul" else F32, tag="r", name=f"r{h}")
                  nc.vector.reciprocal(r_sb[:], po[64:65, :])
                  if BCAST == "gpsimd":
                      rb_sb = work_pool.tile([64, N], F32, tag="rb", name=f"rb{h}")
                      nc.gpsimd.partition_broadcast(rb_sb[:], r_sb[:])
                      nc.vector.tensor_mul(a_sb[r : r + 64, t, :], po[0:64, :], rb_sb[:])
                  else:
                      o_sb = work_pool.tile([64, N], F32, tag="osb", name=f"o{h}")
                      nc.vector.tensor_copy(o_sb[:], po[0:64, :])
                      rb_ps = pp.tile([64, N], F32, tag="s", name=f"rb{h}")
                      for nb in range(2):
                          nc.tensor.matmul(
                              rb_ps[:, nb * 512 : (nb + 1) * 512],
                              ones_sb[0:1, 0:64],
                              r_sb[0:1, nb * 512 : (nb + 1) * 512],
                              start=True,
                              stop=True,
                          )
                      nc.vector.tensor_mul(a_sb[r : r + 64, t, :], o_sb[:], rb_ps[:])

              # ---- phase 3: output projection + bias ----
              for nt in range(NT):
                  py = pp.tile([128, 1024], F32, tag="s", name=f"ps_y{nt}")
                  for kt in range(KT):
                      for j0, j1 in ((0, 512), (512, C)):
                          nc.tensor.matmul(
                              py[:, j0:j1],
                              a_sb[:, kt, nt * 128 : (nt + 1) * 128],
                              wp_sb[:, kt, j0:j1],
                              start=(kt == 0),
                              stop=False,
                          )
                  for j0, j1 in ((0, 512), (512, C)):
                      nc.tensor.matmul(
                          py[:, j0:j1],
                          ones_sb[0:1, :],
                          bi_sb[0:1, j0:j1],
                          start=False,
                          stop=True,
                      )
                  y_sb = work_pool.tile([128, C], F32, tag="y", name=f"y{nt}")
                  nc.vector.tensor_copy(y_sb[:], py[:, 0:C])
                  nc.sync.dma_start(y[nt], y_sb[:])
              p3_pool.release()

            for _ in range(repeat):
                emit_body()

    nc.compile()
    return nc


def make_in_maps(x, prompt, W_qkv, W_proj, b_proj):
    if DT == BF16:
        import ml_dtypes
        cast = lambda a: np.asarray(a, dtype=ml_dtypes.bfloat16)
    else:
        cast = lambda a: np.asarray(a, dtype=np.float32)
    x, prompt, W_qkv, W_proj, b_proj = map(cast, (x, prompt, W_qkv, W_proj, b_proj))
    wq_h = np.ascontiguousarray(W_qkv.T.reshape(KT, 128, 3 * C))
    wp_h = np.ascontiguousarray(W_proj.T.reshape(KT, 128, C))
    bi_h = np.ascontiguousarray(b_proj.reshape(1, C))
    maps = []
    for b in range(B):
        maps.append(
            {
                "xT": np.ascontiguousarray(x[b].T.reshape(KT, 128, N)),
                "wq": wq_h,
                "wp": wp_h,
                "kp": np.ascontiguousarray(prompt[b, 0].transpose(1, 2, 0)),
                "vp": np.ascontiguousarray(prompt[b, 1].transpose(1, 0, 2)),
                "bi": bi_h,
            }
        )
    return maps


def kernel(x, prompt, W_qkv, W_proj, b_proj, **run_kwargs):
    x = np.asarray(x, dtype=np.float32)
    prompt = np.asarray(prompt, dtype=np.float32)
    W_qkv = np.asarray(W_qkv, dtype=np.float32)
    W_proj = np.asarray(W_proj, dtype=np.float32)
    b_proj = np.asarray(b_proj, dtype=np.float32)

    nc = build_nc(repeat=1)
    in_maps = make_in_maps(x, prompt, W_qkv, W_proj, b_proj)
    with _ldw_opt():
        res = run_bass_kernel_spmd(nc, in_maps, core_ids=list(range(B)), **run_kwargs)
    out = np.stack([res.results[b]["y"].reshape(N, C) for b in range(B)])
    if run_kwargs:
        kernel.last_results = res
    return out

